# revision 41
# baseline (speedup 1.0000x reference)
"""Trainium2 Bass kernel for nn_MemoryBlock (scatter_memory).

Mathematical identity: softmax over the memory-unit axis U produces rows
that sum to exactly 1, so the attention path cancels, and the two Linear
layers fold into one (no nonlinearity between them):

    out[b] = relu( s_b @ (Wv Wo)/U + (N/U) bv Wo + bo ),   s_b = sum_n X[b,n,:]

-- a memory-bound column-sum of X (reading 134 MB is the roofline) plus one
tiny 256x128 matmul, with Wvo = (Wv Wo)/U folded on the host.

Sharding: data-parallel over batch B=16 across 8 cores (2 batches/core).

All X descriptors span the full 128 partitions: only that shape gets the
HWDGE's fast port-aligned 16-way engine split (~420 GB/s aggregate);
measured alternatives (120- or 92-partition descriptors) fall into a slow
block-assignment path at half the per-engine rate.

The column-sum runs as 256-wide matmul slices: every 256-aligned slice of
a chunk covers features 0..255, so the row-parity fold happens inside the
PSUM accumulation for free and the result is the finished [1,256] s_b row.

Finale per batch: srow copy(DVE, PSUM->SBUF) -> 2 PE transposes -> stq
copy(DVE) -> PE matmuls (bias via a rank-1 ones x bvo matmul folded into
the accumulation) -> ACT relu straight out of PSUM -> ACT-issued [1,512B]
output DMA (same engine: no handshake).  Batch 0's finale overlaps batch
1's DMA stream, and the last two chunks are 2 rows each so only one small
matmul plus the short finale chain trails the final X byte.

Every HWDGE DMA delivers exactly 16 semaphore increments (idle engine slots
increment immediately, busy ones on completion), so >=16*ndesc waits are
sound fences.  Semaphores are allocated without context managers: the
framework's program-start clear covers them, and skipping the scoped frees
drops ~2us of end-of-program semaphore-clear instructions.
"""

import contextlib

import numpy as np

B, N, FEAT, MEM, U = 16, 8192, 256, 128, 512
NCORES = 8
BPC = B // NCORES

MMW = 512           # matmul moving free width (one PSUM bank of fp32)
RPP = N // 128      # 64 rows per partition per batch
# per batch: 7 x 8-row chunks then 4/2/1/1-row (tiny critical tail: the
# final chunk is one 256-wide matmul)
FCH = [8, 8, 8, 8, 8, 8, 8, 4, 2, 1, 1]
NCH = len(FCH)

_built = None


def _ensure_axon_hooks():
    try:
        import antenv.axon_hooks  # noqa: F401
        return
    except ImportError:
        pass
    import sys
    import types

    m = types.ModuleType("antenv.axon_hooks")
    holder = [None]
    m.set_axon_ntff_profile_hook = lambda h: holder.__setitem__(0, h)
    m.get_axon_ntff_profile_hook = lambda: holder[0]
    sys.modules["antenv.axon_hooks"] = m
    try:
        import antenv

        antenv.axon_hooks = m
    except ImportError:
        pass


def _build():
    import concourse.bacc as bacc
    import concourse.mybir as mybir

    f32 = mybir.dt.float32
    f32r = mybir.dt.float32r
    AF = mybir.ActivationFunctionType
    nc = bacc.Bacc(None, enable_partition_id=False, monotonic_sem_count=0)

    X_d = nc.dram_tensor("Xs", [BPC, N, FEAT], f32r, kind="ExternalInput")
    # host-swizzled: wvo[f, h*128+j] = ((Wv@Wo)/U)[h*128+f, j]
    Wvo_d = nc.dram_tensor("Wvoc", [MEM, 2 * MEM], f32, kind="ExternalInput")
    # (N/U) bv Wo + bo, twice side by side
    bvo_d = nc.dram_tensor("bvoc", [1, BPC * MEM], f32, kind="ExternalInput")
    ones_d = nc.dram_tensor("onesc", [128, 1], f32r, kind="ExternalInput")
    out_d = nc.dram_tensor("outb", [BPC, MEM], f32, kind="ExternalOutput")

    ctx = contextlib.ExitStack()
    with ctx:
        xts = [
            [
                ctx.enter_context(
                    nc.sbuf_tensor(f"xt{b}_{c}", [128, FCH[c] * FEAT], f32r)
                )
                for c in range(NCH)
            ]
            for b in range(BPC)
        ]
        ones = ctx.enter_context(nc.sbuf_tensor("ones", [128, 1], f32r))
        one_f = ctx.enter_context(nc.sbuf_tensor("one_f", [1, 1], f32))
        wvo_sb = ctx.enter_context(nc.sbuf_tensor("wvo_sb", [128, 2 * MEM], f32))
        bvo_sb = ctx.enter_context(nc.sbuf_tensor("bvo_sb", [1, BPC * MEM], f32))
        srow = ctx.enter_context(nc.sbuf_tensor("srow", [1, BPC * 256], f32))
        stq = ctx.enter_context(nc.sbuf_tensor("stq", [128, 2 * BPC], f32))
        res = ctx.enter_context(nc.sbuf_tensor("res", [33, MEM], f32))

        pss = [
            ctx.enter_context(nc.psum_tensor(f"ps{b}", [1, 256], f32))
            for b in range(BPC)
        ]
        pts = ctx.enter_context(nc.psum_tensor("pts", [128, 2 * BPC], f32))
        # batch b's result row lives at partition 32*b (matmul/ACT base rule)
        pres = ctx.enter_context(nc.psum_tensor("pres", [33, MEM], f32))

        # no context managers: skip the scoped end-of-program sem clears
        dsems = [nc.alloc_semaphore(f"dsem{i}") for i in range(BPC * NCH)]
        onesem = nc.alloc_semaphore("onesem")  # ones DMA
        csem = nc.alloc_semaphore("csem")      # wvo+bvo DMAs
        osem = nc.alloc_semaphore("osem")      # output DMAs
        pesem = nc.alloc_semaphore("pesem")    # PE milestones
        vsem = nc.alloc_semaphore("vsem")      # DVE milestones

        coff = [sum(FCH[:c]) * FEAT for c in range(NCH)]

        with nc.Block() as block:

            @block.sync
            def _(sync):
                # X chunk DMAs immediately, in consumption order (FIFO ring
                # -> in-order completion)
                for b in range(BPC):
                    Xb = X_d[b].rearrange("(p r) f -> p (r f)", p=128)
                    for c in range(NCH):
                        w = FCH[c] * FEAT
                        sync.dma_start(
                            out=xts[b][c][:, 0:w],
                            in_=Xb[:, coff[c] : coff[c] + w],
                        ).then_inc(dsems[b * NCH + c], 16)

            @block.scalar
            def _(scalar):
                # consts on the ACT HWDGE ring; ones first (gates first matmul)
                scalar.dma_start(out=ones[:, :], in_=ones_d[:, :]).then_inc(
                    onesem, 16
                )
                scalar.dma_start(out=bvo_sb[:, :], in_=bvo_d[:, :]).then_inc(csem, 16)
                scalar.dma_start(out=wvo_sb[:, :], in_=Wvo_d[:, :]).then_inc(csem, 16)
                # per-batch output DMAs, gated on the DVE relu
                for b in range(BPC):
                    scalar.wait_ge(vsem, 4 if b == 0 else 7)
                    scalar.dma_start(
                        out=out_d[b : b + 1, :], in_=res[32 * b : 32 * b + 1, :]
                    ).then_inc(osem, 16)

            @block.tensor
            def _(pe):
                pe.wait_ge(onesem, 16)
                # 256-wide column-sum slices: every slice covers features
                # 0..255 (row-parity folds away inside the accumulation)
                nmm = sum(FCH) * FEAT // 256   # 66 per batch
                for b in range(BPC):
                    k = 0
                    ins = None
                    for c in range(NCH):
                        pe.wait_ge(dsems[b * NCH + c], 16)
                        for m in range(FCH[c] * FEAT // 256):
                            ins = nc.tensor.matmul(
                                pss[b][:, :],
                                lhsT=ones[:, 0:1],
                                rhs=xts[b][c][:, m * 256 : (m + 1) * 256],
                                start=(k == 0),
                                stop=(k == nmm - 1),
                            )
                            k += 1
                    ins.then_inc(pesem, 1)  # pesem: 3b+1  (colsum done)
                    # open pres row 32b early with the rank-1 bias matmul --
                    # it only needs the consts, so it runs off the tail path
                    if b == 0:
                        pe.wait_ge(csem, 32)
                        pe.wait_ge(vsem, 1)  # one_f memset
                    nc.tensor.matmul(
                        pres[32 * b : 32 * b + 1, :],
                        lhsT=one_f[0:1, 0:1],
                        rhs=bvo_sb[0:1, b * MEM : (b + 1) * MEM],
                        start=True,
                        stop=False,
                    )
                    # transpose folded row (2 x 128) into pts cols 2b+h
                    pe.wait_ge(vsem, 2 if b == 0 else 5)
                    for h in range(2):
                        ins = nc.tensor.matmul(
                            pts[:, 2 * b + h : 2 * b + h + 1],
                            lhsT=srow[
                                0:1, b * 256 + h * 128 : b * 256 + (h + 1) * 128
                            ],
                            rhs=one_f[0:1, 0:1],
                            is_transpose=True,
                            start=True,
                            stop=True,
                        )
                    ins.then_inc(pesem, 1)  # pesem: 3b+2
                    # final matmuls into pres row 32b: 2 stq matmuls
                    pe.wait_ge(vsem, 3 if b == 0 else 6)
                    for h in range(2):
                        ins = nc.tensor.matmul(
                            pres[32 * b : 32 * b + 1, :],
                            lhsT=stq[:, 2 * b + h : 2 * b + h + 1],
                            rhs=wvo_sb[:, h * MEM : (h + 1) * MEM],
                            start=False,
                            stop=(h == 1),
                        )
                    ins.then_inc(pesem, 1)  # pesem: 3b+3

            @block.vector
            def _(vector):
                nc.vector.memset(one_f[:, :], 1.0).then_inc(vsem, 1)  # =1
                for b in range(BPC):
                    # folded colsum row PSUM -> SBUF (transpose lhsT source)
                    vector.wait_ge(pesem, 3 * b + 1)
                    nc.vector.tensor_copy(
                        out=srow[0:1, b * 256 : (b + 1) * 256], in_=pss[b][0:1, :]
                    ).then_inc(vsem, 1)  # =3b+2
                    # transposed cols PSUM -> SBUF for the final matmul lhsT
                    vector.wait_ge(pesem, 3 * b + 2)
                    nc.vector.tensor_copy(
                        out=stq[:, 2 * b : 2 * b + 2],
                        in_=pts[:, 2 * b : 2 * b + 2],
                    ).then_inc(vsem, 1)  # =3b+3
                    # relu straight out of PSUM (cheaper on DVE than ACT)
                    vector.wait_ge(pesem, 3 * b + 3)
                    nc.vector.tensor_scalar_max(
                        out=res[32 * b : 32 * b + 1, :],
                        in0=pres[32 * b : 32 * b + 1, :],
                        scalar1=0.0,
                    ).then_inc(vsem, 1)  # =3b+4

            @block.gpsimd
            def _(gpsimd):
                gpsimd.wait_ge(osem, 32)

    if not nc.is_finalized():
        nc.finalize()
    return nc


def kernel(X, mem, Wk, bk, Wv, bv, Wo, bo):
    global _built
    _ensure_axon_hooks()
    from concourse.bass_utils import run_bass_kernel_spmd

    if _built is None:
        _built = _build()
    nc = _built

    X = np.asarray(X, dtype=np.float32)
    Wv64 = np.asarray(Wv, dtype=np.float64)
    Wo64 = np.asarray(Wo, dtype=np.float64)
    Wvo = ((Wv64 @ Wo64) / float(U)).astype(np.float32)          # [FEAT, MEM]
    # wvo_sb[f, h*128+j] = Wvo[h*128+f, j]
    Wvoc = np.ascontiguousarray(
        Wvo.reshape(2, MEM, MEM).transpose(1, 0, 2).reshape(MEM, 2 * MEM)
    )
    bvo = (
        (float(N) / float(U)) * (np.asarray(bv, np.float64) @ Wo64)
        + np.asarray(bo, np.float64)
    ).astype(np.float32)                                          # [MEM]
    bvoc = np.ascontiguousarray(np.tile(bvo, BPC).reshape(1, BPC * MEM))
    onesc = np.ones((128, 1), dtype=np.float32)

    in_maps = [
        {
            "Xs": np.ascontiguousarray(X[i * BPC : (i + 1) * BPC]),
            "Wvoc": Wvoc,
            "bvoc": bvoc,
            "onesc": onesc,
        }
        for i in range(NCORES)
    ]
    r = run_bass_kernel_spmd(nc, in_maps, list(range(NCORES)))
    kernel._last_results = r

    out = np.empty((B, MEM), dtype=np.float32)
    for i in range(NCORES):
        out[i * BPC : (i + 1) * BPC] = r.results[i]["outb"]
    return out


# revision 42
# speedup vs baseline: 1.0124x; 1.0124x over previous
"""Trainium2 Bass kernel for nn_MemoryBlock (scatter_memory).

Mathematical identity: softmax over the memory-unit axis U produces rows
that sum to exactly 1, so the attention path cancels, and the two Linear
layers fold into one (no nonlinearity between them):

    out[b] = relu( s_b @ (Wv Wo)/U + (N/U) bv Wo + bo ),   s_b = sum_n X[b,n,:]

-- a memory-bound column-sum of X (reading 134 MB is the roofline) plus one
tiny 256x128 matmul, with Wvo = (Wv Wo)/U folded on the host.

Sharding: data-parallel over batch B=16 across 8 cores (2 batches/core).

All X descriptors span the full 128 partitions: only that shape gets the
HWDGE's fast port-aligned 16-way engine split (~420 GB/s aggregate);
measured alternatives (120- or 92-partition descriptors) fall into a slow
block-assignment path at half the per-engine rate.

The column-sum runs as 256-wide matmul slices: every 256-aligned slice of
a chunk covers features 0..255, so the row-parity fold happens inside the
PSUM accumulation for free and the result is the finished [1,256] s_b row.

Finale per batch: srow copy(DVE, PSUM->SBUF) -> 2 PE transposes -> stq
copy(DVE) -> PE matmuls (bias via a rank-1 ones x bvo matmul folded into
the accumulation) -> ACT relu straight out of PSUM -> ACT-issued [1,512B]
output DMA (same engine: no handshake).  Batch 0's finale overlaps batch
1's DMA stream, and the last two chunks are 2 rows each so only one small
matmul plus the short finale chain trails the final X byte.

Every HWDGE DMA delivers exactly 16 semaphore increments (idle engine slots
increment immediately, busy ones on completion), so >=16*ndesc waits are
sound fences.  Semaphores are allocated without context managers: the
framework's program-start clear covers them, and skipping the scoped frees
drops ~2us of end-of-program semaphore-clear instructions.
"""

import contextlib

import numpy as np

B, N, FEAT, MEM, U = 16, 8192, 256, 128, 512
NCORES = 8
BPC = B // NCORES

MMW = 512           # matmul moving free width (one PSUM bank of fp32)
RPP = N // 128      # 64 rows per partition per batch
# per batch: 7 x 8-row chunks then 4/2/1/1-row (tiny critical tail: the
# final chunk is one 256-wide matmul)
FCH = [8, 8, 8, 8, 8, 8, 8, 4, 2, 1, 1]
NCH = len(FCH)

_built = None


def _ensure_axon_hooks():
    try:
        import antenv.axon_hooks  # noqa: F401
        return
    except ImportError:
        pass
    import sys
    import types

    m = types.ModuleType("antenv.axon_hooks")
    holder = [None]
    m.set_axon_ntff_profile_hook = lambda h: holder.__setitem__(0, h)
    m.get_axon_ntff_profile_hook = lambda: holder[0]
    sys.modules["antenv.axon_hooks"] = m
    try:
        import antenv

        antenv.axon_hooks = m
    except ImportError:
        pass


def _build():
    import concourse.bacc as bacc
    import concourse.mybir as mybir

    f32 = mybir.dt.float32
    f32r = mybir.dt.float32r
    AF = mybir.ActivationFunctionType
    nc = bacc.Bacc(None, enable_partition_id=False, monotonic_sem_count=0)

    X_d = nc.dram_tensor("Xs", [BPC, N, FEAT], f32r, kind="ExternalInput")
    # host-swizzled: wvo[f, h*128+j] = ((Wv@Wo)/U)[h*128+f, j]
    Wvo_d = nc.dram_tensor("Wvoc", [MEM, 2 * MEM], f32, kind="ExternalInput")
    # (N/U) bv Wo + bo, twice side by side
    bvo_d = nc.dram_tensor("bvoc", [1, BPC * MEM], f32, kind="ExternalInput")
    ones_d = nc.dram_tensor("onesc", [128, 1], f32r, kind="ExternalInput")
    out_d = nc.dram_tensor("outb", [BPC, MEM], f32, kind="ExternalOutput")

    ctx = contextlib.ExitStack()
    with ctx:
        xts = [
            [
                ctx.enter_context(
                    nc.sbuf_tensor(f"xt{b}_{c}", [128, FCH[c] * FEAT], f32r)
                )
                for c in range(NCH)
            ]
            for b in range(BPC)
        ]
        ones = ctx.enter_context(nc.sbuf_tensor("ones", [128, 1], f32r))
        one_f = ctx.enter_context(nc.sbuf_tensor("one_f", [1, 1], f32))
        wvo_sb = ctx.enter_context(nc.sbuf_tensor("wvo_sb", [128, 2 * MEM], f32))
        bvo_sb = ctx.enter_context(nc.sbuf_tensor("bvo_sb", [1, BPC * MEM], f32))
        srow = ctx.enter_context(nc.sbuf_tensor("srow", [1, BPC * 256], f32))
        stq = ctx.enter_context(nc.sbuf_tensor("stq", [128, 2 * BPC], f32))
        res = ctx.enter_context(nc.sbuf_tensor("res", [33, MEM], f32))

        pss = [
            ctx.enter_context(nc.psum_tensor(f"ps{b}", [1, 256], f32))
            for b in range(BPC)
        ]
        pts = ctx.enter_context(nc.psum_tensor("pts", [128, 2 * BPC], f32))
        # batch b's result row lives at partition 32*b (matmul/ACT base rule)
        pres = ctx.enter_context(nc.psum_tensor("pres", [33, MEM], f32))

        # no context managers: skip the scoped end-of-program sem clears
        dsems = [nc.alloc_semaphore(f"dsem{i}") for i in range(BPC * NCH)]
        onesem = nc.alloc_semaphore("onesem")  # ones DMA
        csem = nc.alloc_semaphore("csem")      # wvo+bvo DMAs
        osem = nc.alloc_semaphore("osem")      # output DMAs
        pesem = nc.alloc_semaphore("pesem")    # PE milestones
        vsem = nc.alloc_semaphore("vsem")      # DVE milestones

        coff = [sum(FCH[:c]) * FEAT for c in range(NCH)]

        with nc.Block() as block:

            @block.sync
            def _(sync):
                # X chunk DMAs immediately, in consumption order (FIFO ring
                # -> in-order completion)
                for b in range(BPC):
                    Xb = X_d[b].rearrange("(p r) f -> p (r f)", p=128)
                    for c in range(NCH):
                        w = FCH[c] * FEAT
                        sync.dma_start(
                            out=xts[b][c][:, 0:w],
                            in_=Xb[:, coff[c] : coff[c] + w],
                        ).then_inc(dsems[b * NCH + c], 16)
                # per-batch output DMAs, gated on the DVE relu; the SP ring
                # is long-drained by now
                for b in range(BPC):
                    sync.wait_ge(vsem, 4 if b == 0 else 7)
                    sync.dma_start(
                        out=out_d[b : b + 1, :], in_=res[32 * b : 32 * b + 1, :]
                    ).then_inc(osem, 16)

            @block.scalar
            def _(scalar):
                # consts on the ACT HWDGE ring; ones first (gates first matmul)
                scalar.dma_start(out=ones[:, :], in_=ones_d[:, :]).then_inc(
                    onesem, 16
                )
                scalar.dma_start(out=bvo_sb[:, :], in_=bvo_d[:, :]).then_inc(csem, 16)
                scalar.dma_start(out=wvo_sb[:, :], in_=Wvo_d[:, :]).then_inc(csem, 16)


            @block.tensor
            def _(pe):
                pe.wait_ge(onesem, 16)
                # 256-wide column-sum slices: every slice covers features
                # 0..255 (row-parity folds away inside the accumulation)
                nmm = sum(FCH) * FEAT // 256   # 66 per batch
                for b in range(BPC):
                    k = 0
                    ins = None
                    for c in range(NCH):
                        pe.wait_ge(dsems[b * NCH + c], 16)
                        for m in range(FCH[c] * FEAT // 256):
                            ins = nc.tensor.matmul(
                                pss[b][:, :],
                                lhsT=ones[:, 0:1],
                                rhs=xts[b][c][:, m * 256 : (m + 1) * 256],
                                start=(k == 0),
                                stop=(k == nmm - 1),
                            )
                            k += 1
                    ins.then_inc(pesem, 1)  # pesem: 3b+1  (colsum done)
                    # open pres row 32b early with the rank-1 bias matmul --
                    # it only needs the consts, so it runs off the tail path
                    if b == 0:
                        pe.wait_ge(csem, 32)
                        pe.wait_ge(vsem, 1)  # one_f memset
                    nc.tensor.matmul(
                        pres[32 * b : 32 * b + 1, :],
                        lhsT=one_f[0:1, 0:1],
                        rhs=bvo_sb[0:1, b * MEM : (b + 1) * MEM],
                        start=True,
                        stop=False,
                    )
                    # transpose folded row (2 x 128) into pts cols 2b+h
                    pe.wait_ge(vsem, 2 if b == 0 else 5)
                    for h in range(2):
                        ins = nc.tensor.matmul(
                            pts[:, 2 * b + h : 2 * b + h + 1],
                            lhsT=srow[
                                0:1, b * 256 + h * 128 : b * 256 + (h + 1) * 128
                            ],
                            rhs=one_f[0:1, 0:1],
                            is_transpose=True,
                            start=True,
                            stop=True,
                        )
                    ins.then_inc(pesem, 1)  # pesem: 3b+2
                    # final matmuls into pres row 32b: 2 stq matmuls
                    pe.wait_ge(vsem, 3 if b == 0 else 6)
                    for h in range(2):
                        ins = nc.tensor.matmul(
                            pres[32 * b : 32 * b + 1, :],
                            lhsT=stq[:, 2 * b + h : 2 * b + h + 1],
                            rhs=wvo_sb[:, h * MEM : (h + 1) * MEM],
                            start=False,
                            stop=(h == 1),
                        )
                    ins.then_inc(pesem, 1)  # pesem: 3b+3

            @block.vector
            def _(vector):
                nc.vector.memset(one_f[:, :], 1.0).then_inc(vsem, 1)  # =1
                for b in range(BPC):
                    # folded colsum row PSUM -> SBUF (transpose lhsT source)
                    vector.wait_ge(pesem, 3 * b + 1)
                    nc.vector.tensor_copy(
                        out=srow[0:1, b * 256 : (b + 1) * 256], in_=pss[b][0:1, :]
                    ).then_inc(vsem, 1)  # =3b+2
                    # transposed cols PSUM -> SBUF for the final matmul lhsT
                    vector.wait_ge(pesem, 3 * b + 2)
                    nc.vector.tensor_copy(
                        out=stq[:, 2 * b : 2 * b + 2],
                        in_=pts[:, 2 * b : 2 * b + 2],
                    ).then_inc(vsem, 1)  # =3b+3
                    # relu straight out of PSUM (cheaper on DVE than ACT)
                    vector.wait_ge(pesem, 3 * b + 3)
                    nc.vector.tensor_scalar_max(
                        out=res[32 * b : 32 * b + 1, :],
                        in0=pres[32 * b : 32 * b + 1, :],
                        scalar1=0.0,
                    ).then_inc(vsem, 1)  # =3b+4

            @block.gpsimd
            def _(gpsimd):
                gpsimd.wait_ge(osem, 32)

    if not nc.is_finalized():
        nc.finalize()
    return nc


def kernel(X, mem, Wk, bk, Wv, bv, Wo, bo):
    global _built
    _ensure_axon_hooks()
    from concourse.bass_utils import run_bass_kernel_spmd

    if _built is None:
        _built = _build()
    nc = _built

    X = np.asarray(X, dtype=np.float32)
    Wv64 = np.asarray(Wv, dtype=np.float64)
    Wo64 = np.asarray(Wo, dtype=np.float64)
    Wvo = ((Wv64 @ Wo64) / float(U)).astype(np.float32)          # [FEAT, MEM]
    # wvo_sb[f, h*128+j] = Wvo[h*128+f, j]
    Wvoc = np.ascontiguousarray(
        Wvo.reshape(2, MEM, MEM).transpose(1, 0, 2).reshape(MEM, 2 * MEM)
    )
    bvo = (
        (float(N) / float(U)) * (np.asarray(bv, np.float64) @ Wo64)
        + np.asarray(bo, np.float64)
    ).astype(np.float32)                                          # [MEM]
    bvoc = np.ascontiguousarray(np.tile(bvo, BPC).reshape(1, BPC * MEM))
    onesc = np.ones((128, 1), dtype=np.float32)

    in_maps = [
        {
            "Xs": np.ascontiguousarray(X[i * BPC : (i + 1) * BPC]),
            "Wvoc": Wvoc,
            "bvoc": bvoc,
            "onesc": onesc,
        }
        for i in range(NCORES)
    ]
    r = run_bass_kernel_spmd(nc, in_maps, list(range(NCORES)))
    kernel._last_results = r

    out = np.empty((B, MEM), dtype=np.float32)
    for i in range(NCORES):
        out[i * BPC : (i + 1) * BPC] = r.results[i]["outb"]
    return out
